# revision 7
# baseline (speedup 1.0000x reference)
"""Expert-parallel MoE FFN kernel for Trainium2 (Bass/Tile).

Problem: per-expert grouped-GEMM FFN
    y[e] = relu(x[e] @ wi[e]) @ wo[e]
with E=8 experts, x:[E,4096,1024] fp32, wi:[E,1024,4096], wo:[E,4096,1024].
Output: [E*4096, 1024] fp32.

Sharding: expert dim E across the 8 NeuronCores (1 expert per core, no
cross-core communication). Each core runs the same SPMD program on its
expert's slabs.

Per-core dataflow (C=4096 tokens, D=1024 d_model, F=4096 d_ff), processed
in token blocks of CB=1024:
  1. x[cblock] is transposed on the PE (128x128 tiles via identity matmul)
     into xT [d-part, c] layout.
  2. mm1: hT[f, c] = relu(wi.T-tile @ xT) accumulated over d chunks; the
     stationary operand is a wi tile [128d x 128f], the moving operand is
     xT [128d x 512c]. Output lands f-on-partitions, which is exactly the
     layout mm2 needs. ReLU is applied by ScalarE on the PSUM->SBUF copy.
  3. mm2: y[c, d] = hT-tile.T @ wo-tile accumulated over all 32 f chunks,
     one PSUM bank per 128-token tile (8 banks, f-contiguous so the PE
     never waits on DMA).
All matmuls use float32r (fp32 read, FP22 multiply, fp32 accumulate):
1 cycle/row at N=512 - same PE rate as bf16 but ~1e-4 relative error.

Weights are streamed (wi+wo re-read once per token block: 4x32MB), x/y
touched once => ~160MB DMA per core, well under the ~900us of PE work.
"""

import numpy as np

P = 128
E = 8
C = 4096
D_MODEL = 1024
D_FF = 4096
CB = 1024  # token block


def build_bass(C=C, D=D_MODEL, F=D_FF, CB=CB):
    import concourse.bacc as bacc
    import concourse.tile as tile
    from concourse import mybir
    from concourse.masks import make_identity

    f32 = mybir.dt.float32
    f32r = mybir.dt.float32r
    relu = mybir.ActivationFunctionType.Relu

    assert C % CB == 0 and CB % 512 == 0 and D % 512 == 0 and F % P == 0
    NB = C // CB  # token blocks
    DCH = D // P  # d_model chunks (contraction of mm1)
    FCH = F // P  # d_ff chunks (contraction of mm2)
    CT = CB // P  # 128-token tiles per block
    CH = CB // 512  # 512-token halves per block (mm1 moving dim)
    DH = D // 512  # 512-wide d_model slices (mm2 moving dim)

    nc = bacc.Bacc("TRN2", target_bir_lowering=False, debug=False)
    x = nc.dram_tensor("x", [C, D], f32, kind="ExternalInput").ap()
    wi = nc.dram_tensor("wi", [D, F], f32, kind="ExternalInput").ap()
    wo = nc.dram_tensor("wo", [F, D], f32, kind="ExternalInput").ap()
    y = nc.dram_tensor("y", [C, D], f32, kind="ExternalOutput").ap()

    wi_r = wi.rearrange("(ko p) f -> p ko f", p=P)  # [128, DCH, F]

    FSS = 2  # f-chunks per wi superslab (1KB DMA packets)
    assert FCH % FSS == 0

    with tile.TileContext(nc) as tc:
        with (
            tc.tile_pool(name="const", bufs=1) as const_pool,
            tc.tile_pool(name="ht", bufs=1) as ht_pool,
            tc.tile_pool(name="xt", bufs=1) as xt_pool,
            tc.tile_pool(name="xs", bufs=2) as xs_pool,
            tc.tile_pool(name="wi", bufs=2) as wi_pool,
            tc.tile_pool(name="wo", bufs=3) as wo_pool,
            tc.tile_pool(name="ys", bufs=2) as ys_pool,
            tc.tile_pool(name="psum", bufs=8, space="PSUM") as psum_pool,
        ):
            ident = const_pool.tile([P, P], f32)
            make_identity(nc, ident[:])

            def ps_tile():
                return psum_pool.tile([P, 512], f32, tag="ps", name="ps")

            for b in range(NB):
                c0 = b * CB

                # --- transpose x[cblock] -> xT[p_d, ko, c] ---
                xT = xt_pool.tile([P, DCH, CB], f32r, tag="xt")
                for ct in range(CT):
                    xs = xs_pool.tile([P, D], f32, tag="xs")
                    nc.sync.dma_start(
                        xs[:], x[c0 + ct * P : c0 + (ct + 1) * P, :]
                    )
                    for kg in range(DCH // 4):
                        pst = ps_tile()
                        for t in range(4):
                            nc.tensor.transpose(
                                pst[:, t * P : (t + 1) * P],
                                xs[:, (kg * 4 + t) * P : (kg * 4 + t + 1) * P],
                                ident[:],
                            )
                        nc.vector.tensor_copy(
                            xT[:, kg * 4 : (kg + 1) * 4, ct * P : (ct + 1) * P],
                            pst[:].rearrange("p (k c) -> p k c", k=4),
                        )

                # --- mm1: hT[f, c] = relu(x @ wi)^T for this block ---
                hT = ht_pool.tile([P, FCH, CB], f32r, tag="ht")
                for fs in range(FCH // FSS):
                    wis = wi_pool.tile([P, DCH, FSS * P], f32r, tag="wi")
                    nc.sync.dma_start(
                        wis[:],
                        wi_r[:, :, fs * FSS * P : (fs + 1) * FSS * P].bitcast(f32r),
                    )
                    for fi in range(FSS):
                        f = fs * FSS + fi
                        for ch in range(CH):
                            ph = ps_tile()
                            for ko in range(DCH):
                                nc.tensor.matmul(
                                    ph[:],
                                    lhsT=wis[:, ko, fi * P : (fi + 1) * P],
                                    rhs=xT[:, ko, ch * 512 : (ch + 1) * 512],
                                    start=(ko == 0),
                                    stop=(ko == DCH - 1),
                                )
                            nc.scalar.activation(
                                hT[:, f, ch * 512 : (ch + 1) * 512], ph[:], relu
                            )

                # --- mm2: y[c, d] = hT.T @ wo, f-contiguous accumulation ---
                for dh in range(DH):
                    pys = [ps_tile() for _ in range(CT)]
                    for f in range(FCH):
                        wos = wo_pool.tile([P, 512], f32r, tag="wo")
                        nc.sync.dma_start(
                            wos[:],
                            wo[
                                f * P : (f + 1) * P, dh * 512 : (dh + 1) * 512
                            ].bitcast(f32r),
                        )
                        for ct in range(CT):
                            nc.tensor.matmul(
                                pys[ct][:],
                                lhsT=hT[:, f, ct * P : (ct + 1) * P],
                                rhs=wos[:],
                                start=(f == 0),
                                stop=(f == FCH - 1),
                            )
                    for ct in range(CT):
                        ysb = ys_pool.tile([P, 512], f32, tag="ys")
                        nc.vector.tensor_copy(ysb[:], pys[ct][:])
                        nc.sync.dma_start(
                            y[
                                c0 + ct * P : c0 + (ct + 1) * P,
                                dh * 512 : (dh + 1) * 512,
                            ],
                            ysb[:],
                        )

    nc.compile()
    return nc


_NC_CACHE = {}


def _get_nc(shape_key):
    if shape_key not in _NC_CACHE:
        _NC_CACHE[shape_key] = build_bass(*shape_key)
    return _NC_CACHE[shape_key]


def kernel(dispatched_states, fused_wi_weight, fused_wo_weight):
    from concourse.bass_utils import run_bass_kernel_spmd

    xs = np.ascontiguousarray(np.asarray(dispatched_states, dtype=np.float32))
    wis = np.ascontiguousarray(np.asarray(fused_wi_weight, dtype=np.float32))
    wos = np.ascontiguousarray(np.asarray(fused_wo_weight, dtype=np.float32))
    e, c, d = xs.shape
    f = wis.shape[2]
    assert (e, c, d, f) == (E, C, D_MODEL, D_FF), (e, c, d, f)

    nc = _get_nc((c, d, f, CB))
    in_maps = [{"x": xs[i], "wi": wis[i], "wo": wos[i]} for i in range(e)]
    res = run_bass_kernel_spmd(nc, in_maps, core_ids=list(range(e)))
    out = np.concatenate([res.results[i]["y"] for i in range(e)], axis=0)
    return out.astype(np.float32)
